# revision 1
# baseline (speedup 1.0000x reference)
"""Trainium2 Bass kernel for the autoregressive LSTM problem (v4).

Model (per reference):
  128 warmup LSTM steps over inputs [B=2048, T=128, F=64], U=512 hidden,
  then 32 autoregressive decode steps through a dense head [U, F].

Strategy:
  - Data parallel over 8 NeuronCores: 256 batch per core, weights
    replicated; transposed [feature, batch] on-chip layout. z^T tiles of
    [128, 256] accumulate in PSUM via out = lhsT.T @ rhs.
  - Hybrid precision exploiting LSTM forget-gate damping: errors injected
    at warm step t decay ~2^-(128-t) by the end of warmup, so warm steps
    t < T_CUT=112 run their h-matmuls in fp8e4 + DoubleRow perf mode
    (2 K-chunks per matmul, measured 4967ns vs 7860ns for the fp32r
    h-sweep), while the last 16 warm steps, all decode steps and the
    dense head run fp32r. Simulated end-to-end max-rel error: 7.3e-5.
  - All weights are shipped pre-scaled by 32 (lifts fp8 values out of
    the subnormal range); the gate activations descale for free via the
    ACT input affine (scale=1/32).
  - b and dense_b are zeros per the spec, so no bias work (decode steps
    are a pure h-recurrence: 32 DR / 64 fp32r MMs, no x-matmul). A
    general bias path via DVE adds on PSUM is compiled only if the
    inputs are ever nonzero.
  - Step 0 skips the h-matmuls and the f*c update entirely (h0=c0=0).
  - Decode folds pred away: z_t = h_{t-1} @ (dense_W @ W_x + W_h);
    h history goes to DRAM and the dense head runs as a final phase.
"""

import numpy as np

B = 2048
T = 128
F = 64
U = 512
OUT_STEPS = 32
N_CORES = 8
BL = B // N_CORES  # per-core batch (= matmul N)
T_CUT = 112        # warm steps < T_CUT use fp8 DoubleRow h-matmuls
WSCALE = 32.0      # weights shipped pre-scaled; ACT descales

_CACHE = {}


def build_nc(t_warm=T, t_dec=OUT_STEPS - 1, bl=BL, reps=None, t_cut=T_CUT,
             with_bias=False, skip_warm=False, skip_dec=False,
             skip_final=False):
    """Build the Bass program. Returns nc.

    reps: if set, wrap the whole compute (steps + dense head) in a hardware
    For_i loop running it `reps` times — timing-only variant used to measure
    device time above the dispatch noise floor.
    """
    import contextlib

    import concourse.bass as bass  # noqa: F401
    import concourse.mybir as mybir
    import concourse.tile as tile
    from concourse import bacc

    f32 = mybir.dt.float32
    f32r = mybir.dt.float32r
    f16 = mybir.dt.float16
    f8 = mybir.dt.float8e4
    AF = mybir.ActivationFunctionType
    DR = mybir.MatmulPerfMode.DoubleRow
    n_out = t_dec + 1
    inv_s = 1.0 / WSCALE

    nc = bacc.Bacc("TRN2", target_bir_lowering=False, debug=False,
                   num_devices=N_CORES)

    # DRAM parameters (per core); weights are pre-scaled by WSCALE
    xT_d = nc.dram_tensor("xT", [t_warm, F, bl], f32r,
                          kind="ExternalInput").ap()
    wx_d = nc.dram_tensor("wx", [F, 4 * U], f32,
                          kind="ExternalInput").ap()
    wh_d = nc.dram_tensor("wh", [U, 4 * U], f32, kind="ExternalInput").ap()
    whd_d = nc.dram_tensor("wh_dec", [U, 4 * U], f32,
                           kind="ExternalInput").ap()
    dw_d = nc.dram_tensor("dense_W", [U, F], f32, kind="ExternalInput").ap()
    out_d = nc.dram_tensor("outT", [n_out, F, bl], f32,
                           kind="ExternalOutput").ap()
    H_d = nc.dram_tensor("H", [n_out, 128, 4 * bl], f32r).ap()
    if with_bias:
        bb_d = nc.dram_tensor("b_bcast", [2, 4, 128, 2 * bl], f32,
                              kind="ExternalInput").ap()
        bbd_d = nc.dram_tensor("b_dec_bcast", [2, 4, 128, 2 * bl], f32,
                               kind="ExternalInput").ap()
        db_d = nc.dram_tensor("dense_b", [F, 1], f32,
                              kind="ExternalInput").ap()

    with tile.TileContext(nc) as tc:
        with (
            tc.tile_pool(name="wpool", bufs=1) as wpool,
            tc.tile_pool(name="state", bufs=1) as state,
        ):
            # ---- load weights; fp32r copies + fp8 copy of wh ----
            with tc.tile_pool(name="staging", bufs=1) as staging:
                wh_f = staging.tile([128, 4, 4 * U], f32, tag="big")
                nc.sync.dma_start(out=wh_f,
                                  in_=wh_d.rearrange("(k p) n -> p k n", p=128))
                wh_sb = wpool.tile([128, 4, 4 * U], f32r)
                nc.vector.tensor_copy(wh_sb, wh_f)
                wh_q8 = wpool.tile([128, 4, 4 * U], f8)
                nc.vector.tensor_copy(wh_q8, wh_f)

                whd_f = staging.tile([128, 4, 4 * U], f32, tag="big2")
                nc.sync.dma_start(out=whd_f,
                                  in_=whd_d.rearrange("(k p) n -> p k n", p=128))
                whd_sb = wpool.tile([128, 4, 4 * U], f32r)
                nc.vector.tensor_copy(whd_sb, whd_f)

                wx_f = staging.tile([F, 4 * U], f32, tag="small")
                nc.sync.dma_start(out=wx_f, in_=wx_d[:, :])
                wx_sb = wpool.tile([F, 4 * U], f32r)
                nc.vector.tensor_copy(wx_sb, wx_f)

                dw_f = staging.tile([128, 4, F], f32, tag="small3")
                nc.sync.dma_start(out=dw_f,
                                  in_=dw_d.rearrange("(k p) n -> p k n", p=128))
                dw_sb = wpool.tile([128, 4, F], f32r)
                nc.vector.tensor_copy(dw_sb, dw_f)

            if with_bias:
                bb_sb = wpool.tile([128, 2, 4, 2 * bl], f32)
                nc.sync.dma_start(
                    out=bb_sb, in_=bb_d.rearrange("h g p n -> p h g n"))
                bbd_sb = wpool.tile([128, 2, 4, 2 * bl], f32)
                nc.sync.dma_start(
                    out=bbd_sb, in_=bbd_d.rearrange("h g p n -> p h g n"))
                db_sb = wpool.tile([F, 1], f32)
                nc.sync.dma_start(out=db_sb, in_=db_d[:, :])

            # ---- persistent state ----
            # h double-buffered by step parity in two families: fp8 (read
            # by DoubleRow steps) and fp32r (read by fp32r steps). Step 0
            # never reads h and the t==0 cell update never reads c, so no
            # memsets are needed.
            c_sb = state.tile([128, 4 * bl], f32)
            h_a = state.tile([128, 4 * bl], f32r)
            h_b = state.tile([128, 4 * bl], f32r)
            h_bufs = [h_a, h_b]
            q_a = state.tile([128, 4 * bl], f8)
            q_b = state.tile([128, 4 * bl], f8)
            q_bufs = [q_a, q_b]

            with (
                tc.tile_pool(name="zps", bufs=1, space="PSUM") as zps,
                tc.tile_pool(name="gates", bufs=3) as gates,
                tc.tile_pool(name="tmp", bufs=6) as tmp,
                tc.tile_pool(name="xf", bufs=6) as xf_pool,
                tc.tile_pool(name="xr", bufs=6) as xr_pool,
                tc.tile_pool(name="hload", bufs=6) as hload,
                tc.tile_pool(name="po", bufs=4) as po,
                tc.For_i(0, reps) if reps else contextlib.nullcontext(),
            ):
                xr_tiles = {}

                def fetch_x(t):
                    # x ships as fp32 bytes bound to an f32r tensor (same
                    # memory format), so it DMAs straight into the matmul
                    # operand tile — no per-step DVE convert on the chain
                    if t >= t_warm:
                        return
                    x_r = xr_pool.tile([F, bl], f32r, tag="xr",
                                       name=f"xr{t}")
                    nc.sync.dma_start(out=x_r, in_=xT_d[t])
                    xr_tiles[t] = x_r

                def step(t, warm):
                    """One LSTM step.

                    z is 8 single-bank PSUM tensors (half x gate); bank
                    (half, g) holds m-tiles m = 4g + 2*half + q for q=0,1.

                    fp8 steps (warm t < t_cut): h-matmuls are DoubleRow
                    pairs (2 K-chunks per MM). Stream: x sweep, then
                    kpair0-A, kpair1-A (half-A banks complete mid-stream),
                    kpair0-B, kpair1-B. fp32r steps: x sweep, k0 sweep,
                    then per-tile (k1,k2,k3) triples, spreading bank
                    completions so the ACT chain overlaps the matmuls.
                    """
                    fp8_step = warm and t < t_cut
                    wh = wh_sb if warm else whd_sb
                    x_r = xr_tiles.pop(t) if warm else None
                    h_rd = h_bufs[t % 2]
                    hq_rd = q_bufs[t % 2]
                    h_wr = h_bufs[(t + 1) % 2]
                    hq_wr = q_bufs[(t + 1) % 2]
                    first = (t == 0)
                    z = [[zps.tile([128, 2 * bl], f32, tag=f"z{half}{g}",
                                   name=f"z{half}{g}_{t}")
                          for g in range(4)] for half in range(2)]

                    def zt(half, g, q):
                        return z[half][g][:, q * bl:(q + 1) * bl]

                    def wsl(half, g, q):
                        m = 4 * g + 2 * half + q
                        return slice(m * 128, (m + 1) * 128)

                    def xmm(half, g, q, start, stop=False):
                        nc.tensor.matmul(
                            zt(half, g, q), wx_sb[:, wsl(half, g, q)],
                            x_r, start=start, stop=stop)

                    def hmm(half, g, q, k, stop=False, start=False):
                        nc.tensor.matmul(
                            zt(half, g, q), wh[:, k, wsl(half, g, q)],
                            h_rd[:, k * bl:(k + 1) * bl],
                            start=start, stop=stop)

                    def hmm8(half, g, q, kk, stop=False, start=False):
                        # DoubleRow: contracts K-chunks 2kk and 2kk+1
                        rhs = hq_rd[:, kk * 2 * bl:(kk + 1) * 2 * bl] \
                            .rearrange("p (two n) -> p two n", two=2)
                        nc.tensor.matmul(
                            zt(half, g, q),
                            wh_q8[:, 2 * kk:2 * kk + 2, wsl(half, g, q)],
                            rhs, start=start, stop=stop, perf_mode=DR)

                    if warm:
                        # t == 0: h is zero so the x-matmuls alone produce
                        # z; the q=1 x-matmul then closes its PSUM group
                        for half in range(2):
                            for g in range(4):
                                for q in range(2):
                                    xmm(half, g, q, start=(q == 0),
                                        stop=(first and q == 1))
                    if not first and fp8_step:
                        for half in range(2):
                            for kk in range(2):
                                for g in range(4):
                                    for q in range(2):
                                        hmm8(half, g, q, kk,
                                             stop=(kk == 1 and q == 1))
                    elif not first:
                        for half in range(2):
                            for g in range(4):
                                for q in range(2):
                                    hmm(half, g, q, 0,
                                        start=(not warm and q == 0))
                        for half in range(2):
                            for g in range(4):
                                for q in range(2):
                                    for k in (1, 2, 3):
                                        hmm(half, g, q, k,
                                            stop=(k == 3 and q == 1))

                    if with_bias:
                        bb = bb_sb if warm else bbd_sb
                        for half in range(2):
                            for g in range(4):
                                nc.vector.tensor_add(z[half][g], z[half][g],
                                                     bb[:, half, g, :])

                    # gate activations (descale by 1/32) + state update
                    i_sb = gates.tile([128, 4 * bl], f32, tag="ig",
                                      name=f"ig{t}")
                    f_sb = None if first else gates.tile(
                        [128, 4 * bl], f32, tag="fg", name=f"fg{t}")
                    g_sb = gates.tile([128, 4 * bl], f32, tag="gg",
                                      name=f"gg{t}")
                    o_sb = gates.tile([128, 4 * bl], f32, tag="og",
                                      name=f"og{t}")
                    for half in range(2):
                        s = slice(half * 2 * bl, (half + 1) * 2 * bl)
                        nc.scalar.activation(i_sb[:, s], z[half][0],
                                             AF.Sigmoid, scale=inv_s)
                        if not first:
                            nc.scalar.activation(f_sb[:, s], z[half][1],
                                                 AF.Sigmoid, scale=inv_s)
                        nc.scalar.activation(g_sb[:, s], z[half][2],
                                             AF.Tanh, scale=inv_s)
                        nc.scalar.activation(o_sb[:, s], z[half][3],
                                             AF.Sigmoid, scale=inv_s)
                        if first:
                            nc.vector.tensor_mul(c_sb[:, s], i_sb[:, s],
                                                 g_sb[:, s])
                        else:
                            t1 = tmp.tile([128, 2 * bl], f32, tag="t1",
                                          name=f"t1_{t}_{half}")
                            nc.vector.tensor_mul(t1, i_sb[:, s], g_sb[:, s])
                            nc.vector.tensor_mul(c_sb[:, s], f_sb[:, s],
                                                 c_sb[:, s])
                            nc.vector.tensor_add(c_sb[:, s], c_sb[:, s], t1)
                        tch = tmp.tile([128, 2 * bl], f32, tag="tc",
                                       name=f"tc_{t}_{half}")
                        nc.scalar.activation(tch, c_sb[:, s], AF.Tanh)
                        if warm and t < t_cut - 1:
                            # next step reads fp8 h only
                            nc.vector.tensor_mul(hq_wr[:, s], o_sb[:, s],
                                                 tch)
                        elif warm and t == t_cut - 1:
                            # transition: next step reads fp32r h
                            nc.vector.tensor_mul(hq_wr[:, s], o_sb[:, s],
                                                 tch)
                            nc.vector.tensor_copy(h_wr[:, s], hq_wr[:, s])
                        else:
                            nc.vector.tensor_mul(h_wr[:, s], o_sb[:, s],
                                                 tch)
                    return h_wr

                # warmup
                h_cur = h_a
                if not skip_warm:
                    fetch_x(0)
                    fetch_x(1)
                    for t in range(t_warm):
                        h_cur = step(t, warm=True)
                        fetch_x(t + 2)
                nc.sync.dma_start(out=H_d[0], in_=h_cur)
                # decode
                if not skip_dec:
                    for t in range(1, t_dec + 1):
                        h_cur = step(t_warm + t - 1, warm=False)
                        nc.sync.dma_start(out=H_d[t], in_=h_cur)

                # final dense phase: pred_t = H[t] @ (32*dense_W) / 32.
                # pred PSUM borrows the z slots (alternating for overlap)
                for t in range(0 if skip_final else n_out):
                    hl = hload.tile([128, 4 * bl], f32r, tag="hl",
                                    name=f"hl{t}")
                    eng = nc.sync if t % 2 == 0 else nc.gpsimd
                    eng.dma_start(out=hl, in_=H_d[t])
                    pps = zps.tile([F, bl], f32, tag=("z00" if t % 2 == 0
                                                      else "z01"),
                                   name=f"pps{t}")
                    for k in range(4):
                        nc.tensor.matmul(pps, dw_sb[:, k, :],
                                         hl[:, k * bl:(k + 1) * bl],
                                         start=(k == 0), stop=(k == 3))
                    p_sb = po.tile([F, bl], f32, tag="po", name=f"po{t}")
                    if with_bias:
                        nc.scalar.activation(p_sb, pps, AF.Identity,
                                             scale=inv_s,
                                             bias=db_sb[:, 0:1])
                    else:
                        nc.scalar.activation(p_sb, pps, AF.Identity,
                                             scale=inv_s)
                    nc.sync.dma_start(out=out_d[t], in_=p_sb)

    nc.compile()
    return nc


def prep_inputs(inputs, W_x, W_h, b, dense_W, dense_b, t_warm=T, bl=BL,
                with_bias=False):
    """Host-side prep: returns per-core input maps (weights x32)."""
    n_cores = inputs.shape[0] // bl
    W_x = np.asarray(W_x, np.float32)
    W_h = np.asarray(W_h, np.float32)
    b = np.asarray(b, np.float32)
    dense_W = np.asarray(dense_W, np.float32)
    dense_b = np.asarray(dense_b, np.float32)

    wh_dec = (W_h.astype(np.float64)
              + dense_W.astype(np.float64) @ W_x.astype(np.float64)
              ).astype(np.float32)
    b_dec = (b.astype(np.float64)
             + dense_b.astype(np.float64) @ W_x.astype(np.float64)
             ).astype(np.float32)

    s = WSCALE
    shared = {
        "wx": (W_x * s).astype(np.float32),
        "wh": (W_h * s).astype(np.float32),
        "wh_dec": (wh_dec * s).astype(np.float32),
        "dense_W": (dense_W * s).astype(np.float32),
    }
    if with_bias:
        # bank (half, g) holds m = 4g + 2*half + q for q = 0, 1:
        # bias value for partition p, column q*bl+n is s*b[m*128+p]
        def bcast(vec):
            out = np.zeros((2, 4, 128, 2 * bl), np.float32)
            for half in range(2):
                for g in range(4):
                    for q in range(2):
                        m = 4 * g + 2 * half + q
                        out[half, g, :, q * bl:(q + 1) * bl] = \
                            s * vec[m * 128:(m + 1) * 128][:, None]
            return out
        shared["b_bcast"] = bcast(b)
        shared["b_dec_bcast"] = bcast(b_dec)
        shared["dense_b"] = dense_b[:, None].astype(np.float32)

    in_maps = []
    x = np.asarray(inputs, np.float32)
    for c in range(n_cores):
        shard = x[c * bl:(c + 1) * bl, :t_warm]          # [bl, t, F]
        xT = np.ascontiguousarray(
            shard.transpose(1, 2, 0).astype(np.float32))  # [t, F, bl]
        in_maps.append({"xT": xT, **shared})
    return in_maps


def gather_output(results, bl=BL):
    """results: list of per-core dicts with outT [n_out, F, bl]."""
    outs = []
    for r in results:
        outs.append(np.ascontiguousarray(r["outT"].transpose(2, 0, 1)))
    return np.concatenate(outs, axis=0)  # [B, out_steps, F]


def kernel(inputs, W_x, W_h, b, dense_W, dense_b):
    from concourse.bass_utils import run_bass_kernel_spmd

    with_bias = bool(np.any(np.asarray(b)) or np.any(np.asarray(dense_b)))
    key = ("nc", with_bias)
    if key not in _CACHE:
        _CACHE[key] = build_nc(with_bias=with_bias)
    nc = _CACHE[key]
    in_maps = prep_inputs(inputs, W_x, W_h, b, dense_W, dense_b,
                          with_bias=with_bias)
    res = run_bass_kernel_spmd(nc, in_maps, core_ids=list(range(N_CORES)),
                               trace=False)
    return gather_output(res.results)



# revision 16
# speedup vs baseline: 1.0944x; 1.0944x over previous
"""Trainium2 Bass kernel for the autoregressive LSTM problem (v5).

Model (per reference):
  128 warmup LSTM steps over inputs [B=2048, T=128, F=64], U=512 hidden,
  then 32 autoregressive decode steps through a dense head [U, F].

Strategy (v5 — ACT-engine-bound redesign):
  - Data parallel over 8 NeuronCores (256 batch/core), further split into
    TWO independent recurrence streams of 128 batch each. The streams
    ping-pong across engines: while stream A's gates run on the
    Activation engine, stream B's matmuls/DVE ops proceed, hiding the
    serial gate-chain latency without hand-scheduling.
  - Gate-unit permutation [i, f, o, g] lets one wide ACT instruction
    compute sigmoid over i|f|o ([128, 1536] from PSUM) per stream per
    step; tanh(g) and tanh(c) are the other two ACT passes. ACT is the
    bottleneck engine at ~5.4us/step-pair (5 transcendental passes).
  - Gates and h in fp16 (2x DVE mode where all operands are 2-byte),
    c stays fp32. Weights ship as fp16 (x32 pre-scale, ACT descales by
    1/32; matmul outputs for the dense head are descaled on the host).
  - Warm steps t < T_CUT=120 use fp8e4+DoubleRow h-matmuls (h and W_h
    quantized on-chip); the last warm steps and decode run fp16
    (simulated end-to-end max-rel err 8.7e-4 vs 2e-2 budget).
  - Decode folds pred away: z_t = h_{t-1} @ (W_h + dense_W @ W_x); h
    history stays in SBUF and the dense head runs as a final phase with
    PSUM->DRAM DMA outputs (host divides by 32).
  - b and dense_b are zeros per the spec; a general bias path is
    compiled only if the inputs are ever nonzero.
"""

import numpy as np

B = 2048
T = 128
F = 64
U = 512
OUT_STEPS = 32
N_CORES = 8
BL = B // N_CORES   # per-core batch
NS = 2              # streams per core
SL = BL // NS       # per-stream batch (= matmul N)
T_CUT = 120         # warm steps < T_CUT use fp8 DoubleRow h-matmuls
WSCALE = 32.0       # weights shipped pre-scaled; ACT / host descale

# permuted gate-unit order [i, f, o, g] so one ACT instr covers i|f|o
PERM = np.concatenate([np.arange(0, 512), np.arange(512, 1024),
                       np.arange(1536, 2048), np.arange(1024, 1536)])

_CACHE = {}


SCHED_DEFAULT = [
    ("ifo", 0, 0), ("g", 0, 0), ("ifo", 1, 0),
    ("c", 0, 0), ("t1", 0, 0), ("ca", 0, 0), ("tc", 0, 0),
    ("g", 1, 0),
    ("c", 0, 1), ("t1", 0, 1), ("ca", 0, 1),
    ("h", 0, 0), ("tc", 0, 1),
    ("c", 1, 0), ("t1", 1, 0), ("ca", 1, 0),
    ("h", 0, 1), ("tc", 1, 0), ("h", 1, 0),
    ("c", 1, 1), ("t1", 1, 1), ("ca", 1, 1),
    ("tc", 1, 1), ("h", 1, 1),
]


def build_nc(t_warm=T, t_dec=OUT_STEPS - 1, reps=None, t_cut=T_CUT,
             dec_fp8=False, h_pool=False, sched=None, with_bias=False,
             skip_warm=False, skip_dec=False, skip_final=False):
    """Build the Bass program. Returns nc.

    reps: if set, wrap the whole compute in a hardware For_i loop running
    it `reps` times — timing-only variant (same contract as v4 test.py).
    """
    import contextlib

    import concourse.bass as bass  # noqa: F401
    import concourse.mybir as mybir
    import concourse.tile as tile
    from concourse import bacc

    f32 = mybir.dt.float32
    f16 = mybir.dt.float16
    f8 = mybir.dt.float8e4
    AF = mybir.ActivationFunctionType
    DR = mybir.MatmulPerfMode.DoubleRow
    n_out = t_dec + 1
    inv_s = 1.0 / WSCALE

    if sched is None:
        sched = SCHED_DEFAULT
    nc = bacc.Bacc("TRN2", target_bir_lowering=False, debug=False,
                   num_devices=N_CORES)

    xT_d = nc.dram_tensor("xT", [max(t_warm, 1), F, BL], f16,
                          kind="ExternalInput").ap()
    wx_d = nc.dram_tensor("wx", [F, 4 * U], f16, kind="ExternalInput").ap()
    wh_d = nc.dram_tensor("wh", [U, 4 * U], f16, kind="ExternalInput").ap()
    whd_d = nc.dram_tensor("wh_dec", [U, 4 * U], f16,
                           kind="ExternalInput").ap()
    dw_d = nc.dram_tensor("dense_W", [U, F], f16, kind="ExternalInput").ap()
    out_d = nc.dram_tensor("outT", [n_out, F, BL], f32,
                           kind="ExternalOutput").ap()
    if with_bias:
        bifo_d = nc.dram_tensor("b_ifo", [128, 12 * SL], f32,
                                kind="ExternalInput").ap()
        bg_d = nc.dram_tensor("b_g", [128, 4 * SL], f32,
                              kind="ExternalInput").ap()
        bifo_dec_d = nc.dram_tensor("b_dec_ifo", [128, 12 * SL], f32,
                                    kind="ExternalInput").ap()
        bg_dec_d = nc.dram_tensor("b_dec_g", [128, 4 * SL], f32,
                                  kind="ExternalInput").ap()
        db_d = nc.dram_tensor("dense_b", [F, 1], f32,
                              kind="ExternalInput").ap()

    with tile.TileContext(nc) as tc:
        with (
            tc.tile_pool(name="wpool", bufs=1) as wpool,
            tc.tile_pool(name="state", bufs=1) as state,
        ):
            # ---- weights: fp16 from host; fp8 copies made on-chip ----
            wh16 = wpool.tile([128, 4, 4 * U], f16)
            nc.sync.dma_start(out=wh16,
                              in_=wh_d.rearrange("(k p) n -> p k n", p=128))
            wh8 = wpool.tile([128, 4, 4 * U], f8)
            nc.vector.tensor_copy(wh8, wh16)

            whd16 = wpool.tile([128, 4, 4 * U], f16)
            nc.sync.dma_start(out=whd16,
                              in_=whd_d.rearrange("(k p) n -> p k n", p=128))
            if dec_fp8:
                whd8 = wpool.tile([128, 4, 4 * U], f8)
                nc.vector.tensor_copy(whd8, whd16)
                whd8r = wpool.tile([128, 4, 4 * U], f8)
                nc.vector.tensor_sub(whd8r, whd16, whd8)

            wx_sb = wpool.tile([F, 4 * U], f16)
            nc.sync.dma_start(out=wx_sb, in_=wx_d[:, :])
            dw_sb = wpool.tile([128, 4, F], f16)
            nc.sync.dma_start(out=dw_sb,
                              in_=dw_d.rearrange("(k p) n -> p k n", p=128))
            if with_bias:
                bifo_sb = wpool.tile([128, 12 * SL], f32)
                nc.sync.dma_start(out=bifo_sb, in_=bifo_d)
                bg_sb = wpool.tile([128, 4 * SL], f32)
                nc.sync.dma_start(out=bg_sb, in_=bg_d)
                bifo_dec_sb = wpool.tile([128, 12 * SL], f32)
                nc.sync.dma_start(out=bifo_dec_sb, in_=bifo_dec_d)
                bg_dec_sb = wpool.tile([128, 4 * SL], f32)
                nc.sync.dma_start(out=bg_dec_sb, in_=bg_dec_d)
                db_sb = wpool.tile([F, 1], f32)
                nc.sync.dma_start(out=db_sb, in_=db_d[:, :])

            # ---- persistent per-stream state ----
            c_st = [state.tile([128, 4 * SL], f32, name=f"c{s}")
                    for s in range(NS)]
            q_bufs = [[state.tile([128, 4 * SL], f8, name=f"hq{s}_{p}")
                       for p in range(2)] for s in range(NS)]
            h_bufs = [[state.tile([128, 4 * SL], f16, name=f"hf{s}_{p}")
                       for p in range(2)] for s in range(NS)]
            # decode h history doubles as the recurrence buffer
            hist = [[state.tile([128, 4 * SL], f16, name=f"H{s}_{j}")
                     for j in range(n_out)] for s in range(NS)]

            with (
                tc.tile_pool(name="zps", bufs=1, space="PSUM") as zps,
                tc.tile_pool(name="gates", bufs=2) as gates,
                tc.tile_pool(name="tmp", bufs=3) as tmp,
                tc.tile_pool(name="xp", bufs=4) as xp,
                tc.For_i(0, reps) if reps else contextlib.nullcontext(),
            ):
                x_tiles = {}

                def fetch_x(t):
                    if t >= t_warm or skip_warm:
                        return
                    x_r = xp.tile([F, BL], f16, tag="x", name=f"x{t}")
                    nc.sync.dma_start(out=x_r, in_=xT_d[t])
                    x_tiles[t] = x_r

                cur = [{} for _ in range(NS)]  # per-stream step tiles

                def step_pe(s, t, warm, hist_j=None):
                    """Matmul phase of one LSTM step for stream s.

                    t: global step index (0..t_warm+t_dec-1). Decode steps
                    pass hist_j to read h from hist[s][hist_j-1] and write
                    hist[s][hist_j].
                    """
                    first = (t == 0)
                    fp8s = warm and t < t_cut
                    zifo = zps.tile([128, 12 * SL], f32, tag=f"zifo{s}",
                                    name=f"zifo{s}_{t}")
                    zg = zps.tile([128, 4 * SL], f32, tag=f"zg{s}",
                                  name=f"zg{s}_{t}")
                    cur[s] = {"zifo": zifo, "zg": zg}

                    def zt(m):
                        if m < 12:
                            return zifo[:, m * SL:(m + 1) * SL]
                        return zg[:, (m - 12) * SL:(m - 11) * SL]

                    # one start/stop per 2KB PSUM bank (= 4 m-slices):
                    # start on the first matmul touching the bank, stop on
                    # the final sweep's last slice
                    if warm:
                        x_rhs = x_tiles[t][:, s * SL:(s + 1) * SL]
                        for m in range(16):
                            nc.tensor.matmul(
                                zt(m), wx_sb[:, m * 128:(m + 1) * 128],
                                x_rhs, start=(m % 4 == 0),
                                stop=(first and m % 4 == 3))
                        if s == NS - 1:
                            x_tiles.pop(t)

                    if not first and fp8s:
                        h8_rd = q_bufs[s][t % 2]
                        for kk in range(2):
                            rhs = h8_rd[:, kk * 2 * SL:(kk + 1) * 2 * SL] \
                                .rearrange("p (two n) -> p two n", two=2)
                            for m in range(16):
                                nc.tensor.matmul(
                                    zt(m),
                                    wh8[:, 2 * kk:2 * kk + 2,
                                        m * 128:(m + 1) * 128],
                                    rhs, start=False,
                                    stop=(kk == 1 and m % 4 == 3),
                                    perf_mode=DR)
                    elif not first and warm:
                        h16_rd = h_bufs[s][t % 2]
                        for k in range(4):
                            rhs = h16_rd[:, k * SL:(k + 1) * SL]
                            for m in range(16):
                                nc.tensor.matmul(
                                    zt(m), wh16[:, k, m * 128:(m + 1) * 128],
                                    rhs, start=False,
                                    stop=(k == 3 and m % 4 == 3))
                    elif not first:
                        # decode: no x-matmul; folded weights
                        if dec_fp8:
                            h8_rd = q_bufs[s][t % 2]
                            for kk in range(2):
                                rhs = h8_rd[:,
                                            kk * 2 * SL:(kk + 1) * 2 * SL] \
                                    .rearrange("p (two n) -> p two n",
                                               two=2)
                                for wt in (whd8, whd8r):
                                    for m in range(16):
                                        nc.tensor.matmul(
                                            zt(m),
                                            wt[:, 2 * kk:2 * kk + 2,
                                               m * 128:(m + 1) * 128],
                                            rhs,
                                            start=(wt is whd8 and kk == 0
                                                   and m % 4 == 0),
                                            stop=(wt is whd8r and kk == 1
                                                  and m % 4 == 3),
                                            perf_mode=DR)
                        else:
                            h16_rd = hist[s][hist_j - 1]
                            for k in range(4):
                                rhs = h16_rd[:, k * SL:(k + 1) * SL]
                                for m in range(16):
                                    nc.tensor.matmul(
                                        zt(m),
                                        whd16[:, k, m * 128:(m + 1) * 128],
                                        rhs,
                                        start=(k == 0 and m % 4 == 0),
                                        stop=(k == 3 and m % 4 == 3))

                    if with_bias:
                        bi = bifo_sb if warm else bifo_dec_sb
                        bg = bg_sb if warm else bg_dec_sb
                        nc.vector.tensor_add(zifo, zifo, bi)
                        nc.vector.tensor_add(zg, zg, bg)

                def step_gates(s, t):
                    # gates: sigmoid over i|f|o in one pass, tanh(g)
                    ifo = gates.tile([128, 12 * SL], f16, tag=f"ifo{s}",
                                     name=f"ifo{s}_{t}")
                    nc.scalar.activation(ifo, cur[s]["zifo"], AF.Sigmoid,
                                         scale=inv_s)
                    g_sb = gates.tile([128, 4 * SL], f16, tag=f"g{s}",
                                      name=f"g{s}_{t}")
                    nc.scalar.activation(g_sb, cur[s]["zg"], AF.Tanh,
                                         scale=inv_s)
                    cur[s]["ifo"] = ifo
                    cur[s]["g"] = g_sb

                def dve_c(s, t, half):
                    # half-width c-chain so tanh(c) half a is ready early
                    if t == 0:
                        return
                    hs = slice(half * 2 * SL, (half + 1) * 2 * SL)
                    f_ = cur[s]["ifo"][:, 4 * SL:8 * SL][:, hs]
                    c = c_st[s][:, hs]
                    nc.vector.tensor_mul(c, f_, c)

                def dve_t1(s, t, half):
                    hs = slice(half * 2 * SL, (half + 1) * 2 * SL)
                    i_ = cur[s]["ifo"][:, 0:4 * SL][:, hs]
                    g_sb = cur[s]["g"]
                    if t == 0:
                        nc.vector.tensor_mul(c_st[s][:, hs], i_, g_sb[:, hs])
                        return
                    if half == 0 or "t1" not in cur[s]:
                        cur[s]["t1"] = tmp.tile([128, 4 * SL], f16,
                                                tag=f"t1{s}", name=f"t1{s}_{t}")
                    nc.vector.tensor_mul(cur[s]["t1"][:, hs], i_,
                                         g_sb[:, hs])

                def dve_cadd(s, t, half):
                    if t == 0:
                        return
                    hs = slice(half * 2 * SL, (half + 1) * 2 * SL)
                    c = c_st[s][:, hs]
                    nc.vector.tensor_add(c, c, cur[s]["t1"][:, hs])

                def step_tanh_c(s, t, half):
                    # halves a/b so h chunks 0-1 land before 2-3 and the
                    # next step's kk0 DR sweep starts early
                    if half == 0:
                        tch = tmp.tile([128, 4 * SL], f16, tag=f"tch{s}",
                                       name=f"tch{s}_{t}")
                        cur[s]["tch"] = tch
                    tch = cur[s]["tch"]
                    hs = slice(half * 2 * SL, (half + 1) * 2 * SL)
                    nc.scalar.activation(tch[:, hs], c_st[s][:, hs], AF.Tanh)

                def step_h(s, t, warm, half, hist_j=None):
                    # h write: pick the consumer's dtype/location
                    eng = nc.gpsimd if h_pool else nc.vector
                    hs = slice(half * 2 * SL, (half + 1) * 2 * SL)
                    o_ = cur[s]["ifo"][:, 8 * SL:12 * SL][:, hs]
                    tch = cur[s]["tch"][:, hs]
                    last_warm = warm and (t == t_warm - 1)
                    if last_warm:
                        eng.tensor_mul(hist[s][0][:, hs], o_, tch)
                        if dec_fp8 and t_dec > 0:
                            eng.tensor_mul(
                                q_bufs[s][(t + 1) % 2][:, hs], o_, tch)
                    elif warm and (t + 1 < t_cut):
                        eng.tensor_mul(q_bufs[s][(t + 1) % 2][:, hs],
                                       o_, tch)
                    elif warm:
                        eng.tensor_mul(h_bufs[s][(t + 1) % 2][:, hs],
                                       o_, tch)
                    else:
                        eng.tensor_mul(hist[s][hist_j][:, hs], o_, tch)
                        if dec_fp8 and hist_j < t_dec:
                            eng.tensor_mul(
                                q_bufs[s][(t + 1) % 2][:, hs], o_, tch)

                def step_gates_ifo(s, t):
                    ifo = gates.tile([128, 12 * SL], f16, tag=f"ifo{s}",
                                     name=f"ifo{s}_{t}")
                    nc.scalar.activation(ifo, cur[s]["zifo"], AF.Sigmoid,
                                         scale=inv_s)
                    cur[s]["ifo"] = ifo

                def step_gates_g(s, t):
                    g_sb = gates.tile([128, 4 * SL], f16, tag=f"g{s}",
                                      name=f"g{s}_{t}")
                    nc.scalar.activation(g_sb, cur[s]["zg"], AF.Tanh,
                                         scale=inv_s)
                    cur[s]["g"] = g_sb

                def step_all(t, warm, hist_j=None):
                    # emission order doubles as engine-queue priority; see
                    # SCHED: each entry is (kind, stream, half)
                    for s in range(NS):
                        step_pe(s, t, warm, hist_j)
                    for kind, s, half in sched:
                        if kind == "ifo":
                            step_gates_ifo(s, t)
                        elif kind == "g":
                            step_gates_g(s, t)
                        elif kind == "c":
                            dve_c(s, t, half)
                        elif kind == "t1":
                            dve_t1(s, t, half)
                        elif kind == "ca":
                            dve_cadd(s, t, half)
                        elif kind == "tc":
                            step_tanh_c(s, t, half)
                        elif kind == "h":
                            step_h(s, t, warm, half, hist_j)

                # ---- warm phase ----
                if not skip_warm:
                    for t in range(min(4, t_warm)):
                        fetch_x(t)
                    for t in range(t_warm):
                        step_all(t, warm=True)
                        fetch_x(t + 4)
                elif not skip_dec or not skip_final:
                    for s in range(NS):
                        nc.gpsimd.memset(hist[s][0], 0.0)
                        nc.gpsimd.memset(c_st[s], 0.0)
                        if dec_fp8:
                            nc.gpsimd.memset(q_bufs[s][t_warm % 2], 0.0)

                # ---- decode phase ----
                if not skip_dec:
                    for j in range(1, t_dec + 1):
                        step_all(t_warm + j - 1, warm=False, hist_j=j)

                # ---- final dense phase: pred_j = hist[j] @ (32*dense_W),
                # host divides by 32; PSUM goes straight to DRAM ----
                if not skip_final:
                    for j in range(n_out):
                        for s in range(NS):
                            tag = f"zifo{s}" if j % 2 else f"zg{s}"
                            pp = zps.tile([F, SL], f32, tag=tag,
                                          name=f"pp{s}_{j}")
                            for k in range(4):
                                nc.tensor.matmul(
                                    pp, dw_sb[:, k, :],
                                    hist[s][j][:, k * SL:(k + 1) * SL],
                                    start=(k == 0), stop=(k == 3))
                            p_sb = tmp.tile([F, SL], f32, tag=f"po{s}",
                                            name=f"po{s}_{j}")
                            if with_bias:
                                nc.scalar.activation(p_sb, pp, AF.Identity,
                                                     bias=db_sb[:, 0:1])
                            elif s % 2 == 0:
                                nc.scalar.copy(p_sb, pp)
                            else:
                                nc.vector.tensor_copy(p_sb, pp)
                            eng = nc.gpsimd if (j + s) % 2 else nc.sync
                            eng.dma_start(
                                out=out_d[j, :, s * SL:(s + 1) * SL],
                                in_=p_sb)

    nc.compile()
    return nc


def prep_inputs(inputs, W_x, W_h, b, dense_W, dense_b, t_warm=T,
                with_bias=False):
    """Host-side prep: returns per-core input maps (weights x32, fp16,
    gate-units permuted to [i, f, o, g])."""
    n_cores = inputs.shape[0] // BL
    W_x = np.asarray(W_x, np.float32)
    W_h = np.asarray(W_h, np.float32)
    b = np.asarray(b, np.float32)
    dense_W = np.asarray(dense_W, np.float32)
    dense_b = np.asarray(dense_b, np.float32)

    wh_dec = (W_h.astype(np.float64)
              + dense_W.astype(np.float64) @ W_x.astype(np.float64)
              ).astype(np.float32)
    b_dec = (b.astype(np.float64)
             + dense_b.astype(np.float64) @ W_x.astype(np.float64)
             ).astype(np.float32)

    s = WSCALE
    shared = {
        "wx": (W_x * s)[:, PERM].astype(np.float16),
        "wh": (W_h * s)[:, PERM].astype(np.float16),
        "wh_dec": (wh_dec * s)[:, PERM].astype(np.float16),
        "dense_W": (dense_W * s).astype(np.float16),
    }
    if with_bias:
        def bcast(vec):
            # permuted unit m*128+p broadcast across SL batch cols
            vp = (s * vec)[PERM].reshape(16, 128)  # [m, p]
            full = np.repeat(vp[:, :, None], SL, axis=2)  # [m, p, n]
            ifo = full[:12].transpose(1, 0, 2).reshape(128, 12 * SL)
            g = full[12:].transpose(1, 0, 2).reshape(128, 4 * SL)
            return (np.ascontiguousarray(ifo, dtype=np.float32),
                    np.ascontiguousarray(g, dtype=np.float32))
        bifo, bg = bcast(b)
        bifo_dec, bg_dec = bcast(b_dec)
        shared["b_ifo"] = bifo
        shared["b_g"] = bg
        shared["b_dec_ifo"] = bifo_dec
        shared["b_dec_g"] = bg_dec
        shared["dense_b"] = (s * dense_b)[:, None].astype(np.float32)

    in_maps = []
    x = np.asarray(inputs, np.float32)
    for cix in range(n_cores):
        shard = x[cix * BL:(cix + 1) * BL, :t_warm]       # [BL, t, F]
        xT = np.ascontiguousarray(
            shard.transpose(1, 2, 0).astype(np.float16))  # [t, F, BL]
        in_maps.append({"xT": xT, **shared})
    return in_maps


def gather_output(results):
    """results: list of per-core dicts with outT [n_out, F, BL] (x32)."""
    outs = []
    for r in results:
        o = np.ascontiguousarray(r["outT"].transpose(2, 0, 1))
        outs.append((o * (1.0 / WSCALE)).astype(np.float32))
    return np.concatenate(outs, axis=0)  # [B, out_steps, F]


def kernel(inputs, W_x, W_h, b, dense_W, dense_b):
    from concourse.bass_utils import run_bass_kernel_spmd

    with_bias = bool(np.any(np.asarray(b)) or np.any(np.asarray(dense_b)))
    key = ("nc", with_bias)
    if key not in _CACHE:
        _CACHE[key] = build_nc(with_bias=with_bias)
    nc = _CACHE[key]
    in_maps = prep_inputs(inputs, W_x, W_h, b, dense_W, dense_b,
                          with_bias=with_bias)
    res = run_bass_kernel_spmd(nc, in_maps, core_ids=list(range(N_CORES)),
                               trace=False)
    return gather_output(res.results)
